# revision 1
# baseline (speedup 1.0000x reference)
"""Multi-head attention (B=2, S=2048, D=1024, H=16) on 8 NeuronCores.

Sharding: batch x head-group (2 batches x 4 groups of 4 heads). Each core:
  - projects its group's Q^T/K^T (f32r, [256, 2048]) and V (fp16, [2048, 256])
  - attention per head-pair: scores^T via row-packed f32r matmuls,
    exp on ScalarE (fp16 out), attn@V col-packed fp16 matmuls + ones-column
    rowsums, softmax normalization via reciprocal + K=2 selector broadcast
  - partial output projection y_g^T = Wo[:, g] @ out_g^T (fp16 matmuls)
Host: y[b] = sum_g y_g^T.T + bv @ Wo.T + bo.  K-bias drops out of softmax
(per-row constant); Q-bias applied on device; V-bias commutes through the
attention average (rows of attn sum to 1) and is folded host-side.
"""
import numpy as np

B = 2
S = 2048
D = 1024
H = 16
DK = 64
G = 4              # head-groups (cores per batch)
HG = H // G        # heads per group = 4
DH = HG * DK       # group dims = 256
NQB = S // 512     # query blocks
NKC = S // 128     # key chunks
KCD = D // 128     # d_model chunks

_CACHE = {}


def _build_nc():
    import concourse.tile as tile
    import concourse.bacc as bacc
    from concourse import mybir
    from contextlib import ExitStack

    F32R = mybir.dt.float32r
    F32 = mybir.dt.float32
    F16 = mybir.dt.float16
    Exp = mybir.ActivationFunctionType.Exp
    Identity = mybir.ActivationFunctionType.Identity

    nc = bacc.Bacc("TRN2", target_bir_lowering=False, debug=False)

    xq_d = nc.dram_tensor("xq", [D, S], F16, kind="ExternalInput").ap()
    xk_d = nc.dram_tensor("xk", [D, S], F16, kind="ExternalInput").ap()
    xv_d = nc.dram_tensor("xv", [D, S], F16, kind="ExternalInput").ap()
    wq_d = nc.dram_tensor("wq", [D, DH], F16, kind="ExternalInput").ap()
    wk_d = nc.dram_tensor("wk", [D, DH], F16, kind="ExternalInput").ap()
    wv_d = nc.dram_tensor("wv", [D, DH], F16, kind="ExternalInput").ap()
    wo_d = nc.dram_tensor("wo", [DH, D], F16, kind="ExternalInput").ap()
    bq_d = nc.dram_tensor("bq", [128, 2], F32, kind="ExternalInput").ap()
    sel_d = nc.dram_tensor("sel", [64, 128], F32, kind="ExternalInput").ap()
    zr_d = nc.dram_tensor("zr", [64, 512], F32, kind="ExternalInput").ap()
    ones_d = nc.dram_tensor("ones", [128, 1], F16, kind="ExternalInput").ap()
    y_d = nc.dram_tensor("y", [D, S], F32, kind="ExternalOutput").ap()

    with tile.TileContext(nc) as tc, ExitStack() as ctx:
        sbw = ctx.enter_context(tc.tile_pool(name="sbw", bufs=1))
        sbx = ctx.enter_context(tc.tile_pool(name="sbx", bufs=1))
        sbd = ctx.enter_context(tc.tile_pool(name="sbd", bufs=1))
        sbe = ctx.enter_context(tc.tile_pool(name="sbe", bufs=1))
        sbo = ctx.enter_context(tc.tile_pool(name="sbo", bufs=1))
        ps = ctx.enter_context(tc.tile_pool(name="ps", bufs=1, space="PSUM"))

        # ---- weights / constants ------------------------------------------
        # wq/wk/wv: [D, DH]; d_model chunk kc at cols [kc*DH : (kc+1)*DH]
        wq_t = sbw.tile([128, KCD * DH], F16)
        wk_t = sbw.tile([128, KCD * DH], F16)
        wv_t = sbw.tile([128, KCD * DH], F16)
        for kc in range(KCD):
            nc.sync.dma_start(wq_t[:, kc * DH:(kc + 1) * DH], wq_d[kc * 128:(kc + 1) * 128, :])
            nc.sync.dma_start(wk_t[:, kc * DH:(kc + 1) * DH], wk_d[kc * 128:(kc + 1) * 128, :])
            nc.sync.dma_start(wv_t[:, kc * DH:(kc + 1) * DH], wv_d[kc * 128:(kc + 1) * 128, :])
        # wo: [DH, D]; chunk kc2 at cols [kc2*D : (kc2+1)*D]
        wo_t = sbw.tile([128, 2 * D], F16)
        for kc2 in range(2):
            nc.sync.dma_start(wo_t[:, kc2 * D:(kc2 + 1) * D], wo_d[kc2 * 128:(kc2 + 1) * 128, :])
        bq_t = sbw.tile([128, 2], F32)
        nc.sync.dma_start(bq_t[:], bq_d)
        sel_t = sbw.tile([64, 128], F32)
        nc.sync.dma_start(sel_t[:], sel_d)
        recip = sbw.tile([64, 512], F32)
        nc.sync.dma_start(recip[:], zr_d)
        ones_t = sbw.tile([128, 1], F16)
        nc.sync.dma_start(ones_t[:], ones_d)

        # ---- projection outputs -------------------------------------------
        qt_t = [sbd.tile([128, S], F16, name=f"qt{p}") for p in range(2)]
        kt_t = [sbd.tile([128, S], F16, name=f"kt{p}") for p in range(2)]
        v_t = sbd.tile([128, NKC * DH], F16)        # key chunk tb at cols [tb*DH:(tb+1)*DH]
        outsc = [sbd.tile([128, S], F16, name=f"outsc{p}") for p in range(2)]

        def load_x(x_d, dt):
            tiles = []
            for kc in range(KCD):
                t = sbx.tile([128, S], dt, name="xin", tag="xin", bufs=10)
                nc.sync.dma_start(t[:], x_d[kc * 128:(kc + 1) * 128, :])
                tiles.append(t)
            return tiles

        # K^T projection: K^T[pb] = sum_kc wk[kc,pb].T @ xk[kc]
        xk_t = load_x(xk_d, F16)
        for pb in range(2):
            for qb in range(NQB):
                acc = ps.tile([128, 512], F32, name="pacc", tag="scores", bufs=2)
                for kc in range(KCD):
                    nc.tensor.matmul(
                        acc[:],
                        wk_t[:, kc * DH + pb * 128:kc * DH + (pb + 1) * 128],
                        xk_t[kc][:, qb * 512:(qb + 1) * 512],
                        start=(kc == 0), stop=(kc == KCD - 1))
                with nc.allow_low_precision(reason="f32r scores"):
                    nc.vector.tensor_copy(kt_t[pb][:, qb * 512:(qb + 1) * 512], acc[:])

        # V projection: V[tb] = sum_kc xv[kc, tb].T @ wv[kc]   -> fp16
        xv_t = load_x(xv_d, F16)
        for tb in range(NKC):
            acc = ps.tile([128, DH], F32, name="vacc", tag="scores", bufs=2)
            for kc in range(KCD):
                nc.tensor.matmul(
                    acc[:],
                    xv_t[kc][:, tb * 128:(tb + 1) * 128],
                    wv_t[:, kc * DH:(kc + 1) * DH],
                    start=(kc == 0), stop=(kc == KCD - 1))
            with nc.allow_low_precision(reason="fp16 attn weights"):
                nc.vector.tensor_copy(v_t[:, tb * DH:(tb + 1) * DH], acc[:])

        # Q^T projection (with bias) interleaved with pair-0 attention
        xq_t = load_x(xq_d, F16)

        def q_proj(qb):
            for pb in range(2):
                acc = ps.tile([128, 512], F32, name="qacc", tag="scores", bufs=2)
                for kc in range(KCD):
                    nc.tensor.matmul(
                        acc[:],
                        wq_t[:, kc * DH + pb * 128:kc * DH + (pb + 1) * 128],
                        xq_t[kc][:, qb * 512:(qb + 1) * 512],
                        start=(kc == 0), stop=(kc == KCD - 1))
                with nc.allow_low_precision(reason="fp16 scores"):
                    nc.vector.tensor_scalar_add(qt_t[pb][:, qb * 512:(qb + 1) * 512],
                                                acc[:], bq_t[:, pb:pb + 1])

        # ---- output projection for one query block (emitted inside pair-1
        # attention so its dense matmul bursts keep the PE array warm)
        def p3(qb):
            for ypb in range(D // 128):
                yacc = ps.tile([128, 512], F32, name="yacc", tag="scores", bufs=2)
                for kc2 in range(2):
                    nc.tensor.matmul(
                        yacc[:],
                        wo_t[:, kc2 * D + ypb * 128:kc2 * D + (ypb + 1) * 128],
                        outsc[kc2][:, qb * 512:(qb + 1) * 512],
                        start=(kc2 == 0), stop=(kc2 == 1))
                ysb = sbo.tile([128, 512], F32, name="ysb", tag="ysb", bufs=3)
                nc.vector.tensor_copy(ysb[:], yacc[:])
                nc.sync.dma_start(y_d[ypb * 128:(ypb + 1) * 128, qb * 512:(qb + 1) * 512],
                                  ysb[:])

        # ---- attention per head-pair --------------------------------------
        def norm(pair, qb, outA, outB, rs):
            # early psum evacuation: free rs/outA/outB with plain copies,
            # then normalize in SBUF off the PE critical path
            nc.vector.tensor_copy(recip[0:1, :], rs[0:1, :])
            nc.vector.tensor_copy(recip[32:33, :], rs[32:33, :])
            oab = sbo.tile([128, 512], F32, name="oab", tag="oab", bufs=2)
            nc.vector.tensor_copy(oab[0:64, :], outA[0:64, :])
            nc.vector.tensor_copy(oab[64:128, :], outB[64:128, :])
            bc_ps = ps.tile([128, 512], F32, name="bc", tag="scores", bufs=2)
            nc.tensor.matmul(bc_ps[:], sel_t[:], recip[:], start=True, stop=True)
            bc_sb = sbo.tile([128, 512], F32, name="bc_sb", tag="bcastr", bufs=2)
            nc.vector.reciprocal_approx_fast(bc_sb[:], bc_ps[:])
            with nc.allow_low_precision(reason="fp16 out"):
                nc.vector.tensor_mul(outsc[pair][0:64, qb * 512:(qb + 1) * 512],
                                     oab[0:64, :], bc_sb[0:64, :])
                nc.vector.tensor_mul(outsc[pair][64:128, qb * 512:(qb + 1) * 512],
                                     oab[64:128, :], bc_sb[64:128, :])
            if pair == 1:
                p3(qb)

        pend = None
        for pair in range(2):
            ktp, qtp = kt_t[pair], qt_t[pair]
            for qb in range(NQB):
                if pair == 0:
                    q_proj(qb)
                outA = ps.tile([128, 512], F32, name="outA", tag="outA", bufs=1)
                outB = ps.tile([128, 512], F32, name="outB", tag="outB", bufs=1)
                rs = ps.tile([128, 512], F32, name="rs", tag="rs", bufs=1)
                vbase = pair * 128
                ets = {}

                def attn_v(kc):
                    et = ets.pop(kc)
                    nc.tensor.matmul(outA[0:64, :],
                                     v_t[:, kc * DH + vbase:kc * DH + vbase + 64],
                                     et[:, 0:512],
                                     start=(kc == 0), stop=(kc == NKC - 1))
                    nc.tensor.matmul(outB[64:128, :],
                                     v_t[:, kc * DH + vbase + 64:kc * DH + vbase + 128],
                                     et[:, 512:1024],
                                     start=(kc == 0), stop=(kc == NKC - 1))
                    nc.tensor.matmul(rs[0:1, :], ones_t[:], et[:, 0:512],
                                     start=(kc == 0), stop=(kc == NKC - 1))
                    nc.tensor.matmul(rs[32:33, :], ones_t[:], et[:, 512:1024],
                                     start=(kc == 0), stop=(kc == NKC - 1),
                                     skip_group_check=True)

                # software-pipelined: scores/exp for kc are emitted before
                # attn@V/rowsums for kc-1 so the PE never waits on the exp
                for kc in range(NKC):
                    sc = ps.tile([128, 1024], F32, name="sc", tag="scores", bufs=2)
                    nc.tensor.matmul(sc[:, 0:512],
                                     ktp[0:64, kc * 128:(kc + 1) * 128],
                                     qtp[0:64, qb * 512:(qb + 1) * 512],
                                     start=True, stop=True)
                    nc.tensor.matmul(sc[:, 512:1024],
                                     ktp[64:128, kc * 128:(kc + 1) * 128],
                                     qtp[64:128, qb * 512:(qb + 1) * 512],
                                     start=True, stop=True)
                    et = sbe.tile([128, 1024], F16, name="et", tag="et", bufs=4)
                    ets[kc] = et
                    with nc.allow_low_precision(reason="fp16 attn weights"):
                        nc.scalar.activation(et[:], sc[:], Exp, scale=0.125)
                    if kc == 2 and pend is not None:
                        norm(*pend)
                        pend = None
                    if kc > 0:
                        attn_v(kc - 1)
                attn_v(NKC - 1)
                pend = (pair, qb, outA, outB, rs)

        norm(*pend)

    nc.compile()
    return nc


def _get_nc():
    if "nc" not in _CACHE:
        _CACHE["nc"] = _build_nc()
    return _CACHE["nc"]


def kernel(q, k, v, Wq, bq, Wk, bk, Wv, bv, Wo, bo, _trace=False, _tmpdir=None):
    from concourse.bass_utils import run_bass_kernel_spmd

    q = np.asarray(q, np.float32)
    k = np.asarray(k, np.float32)
    v = np.asarray(v, np.float32)
    Wq = np.asarray(Wq, np.float32)
    Wk = np.asarray(Wk, np.float32)
    Wv = np.asarray(Wv, np.float32)
    Wo = np.asarray(Wo, np.float32)
    bq = np.asarray(bq, np.float32)
    bk = np.asarray(bk, np.float32)
    bv = np.asarray(bv, np.float32)
    bo = np.asarray(bo, np.float32)

    nc = _get_nc()

    sel = np.zeros((64, 128), np.float32)
    sel[0, 0:64] = 1.0
    sel[32, 64:128] = 1.0
    zr = np.zeros((64, 512), np.float32)
    ones = np.ones((128, 1), np.float16)

    xT = {}
    for b in range(B):
        xT[("q", b)] = np.ascontiguousarray(q[b].T).astype(np.float16)
        xT[("k", b)] = np.ascontiguousarray(k[b].T).astype(np.float16)
        xT[("v", b)] = np.ascontiguousarray(v[b].T).astype(np.float16)

    # Effective K weights: bk drops out of softmax entirely (adds a
    # per-query-row constant to the scores).  Q bias applied on device.
    in_maps = []
    for c in range(8):
        b, g = c // G, c % G
        gr = slice(g * DH, (g + 1) * DH)
        in_maps.append({
            "xq": xT[("q", b)],
            "xk": xT[("k", b)],
            "xv": xT[("v", b)],
            "wq": np.ascontiguousarray(Wq[gr, :].T).astype(np.float16),
            "wk": np.ascontiguousarray(Wk[gr, :].T).astype(np.float16),
            "wv": np.ascontiguousarray(Wv[gr, :].T).astype(np.float16),
            "wo": np.ascontiguousarray(Wo[:, gr].T).astype(np.float16),
            "bq": np.ascontiguousarray(bq[gr].reshape(2, 128).T),
            "sel": sel,
            "zr": zr,
            "ones": ones,
        })

    kwargs = {}
    if _trace:
        kwargs = dict(trace=True, tmpdir=_tmpdir)
    res = run_bass_kernel_spmd(nc, in_maps, core_ids=list(range(8)), **kwargs)

    # host reduce: y[b] = sum_g y_g^T.T  (+ bias terms folded host-side)
    bias_row = bv @ Wo.T + bo                     # [D]
    out = np.empty((B, S, D), np.float32)
    for b in range(B):
        acc = np.zeros((S, D), np.float32)
        for g in range(G):
            acc += res.results[b * G + g]["y"].T
        out[b] = acc + bias_row[None, :]
    if _trace:
        out = (out, res)
    return out



# revision 19
# speedup vs baseline: 1.1436x; 1.1436x over previous
"""Multi-head attention (B=2, S=2048, D=1024, H=16) on 8 NeuronCores.

Sharding: batch x head-group (2 batches x 4 groups of 4 heads). Each core:
  - projects its group's Q^T/K^T (fp16, [256, 2048]) and V (fp16, [2048, 256])
  - attention per head-pair: scores^T via row-tiled fp16 matmuls, exp on
    ScalarE (fp16 out), attn@V with a ones-column fused into the V stationary
    so PSUM row 64 accumulates the softmax denominator for free
  - normalization via f32r selector-broadcast matmul + fast reciprocal
  - partial output projection y_g^T = Wo[:, g] @ out_g^T (fp16 in/out)
Host: y[b] = sum_g y_g^T.T + bv @ Wo.T + bo.  K-bias drops out of softmax;
Q-bias applied on device; V-bias folded host-side.

Scheduling: the attention inner loop is ScalarE(exp)-bound (~1.15us per
128-key chunk-slot).  All other PE work (K/V/Q projections for later blocks,
output projection, normalization) is fed from a persistent filler queue, one
step per chunk slot, so it hides in the exp slack instead of stalling ScalarE
between query blocks.  Input DMAs are column-block pieces on two queues in
consumption order so compute starts ~8us in.
"""
import numpy as np

B = 2
S = 2048
D = 1024
H = 16
DK = 64
G = 4              # head-groups (cores per batch)
HG = H // G        # heads per group = 4
DH = HG * DK       # group dims = 256
NQB = S // 512     # query blocks
NKC = S // 128     # key chunks
KCD = D // 128     # d_model chunks
LAG = 3            # attn@V trails scores/exp by this many chunk slots

_CACHE = {}


def _build_nc():
    import concourse.tile as tile
    import concourse.bacc as bacc
    from concourse import mybir
    from contextlib import ExitStack

    F32 = mybir.dt.float32
    F32R = mybir.dt.float32r
    F16 = mybir.dt.float16
    Exp = mybir.ActivationFunctionType.Exp

    nc = bacc.Bacc("TRN2", target_bir_lowering=False, debug=False)

    xq_d = nc.dram_tensor("xq", [D, S], F16, kind="ExternalInput").ap()
    xk_d = nc.dram_tensor("xk", [D, S], F16, kind="ExternalInput").ap()
    xv_d = nc.dram_tensor("xv", [D, S], F16, kind="ExternalInput").ap()
    wq_d = nc.dram_tensor("wq", [D, DH], F16, kind="ExternalInput").ap()
    wk_d = nc.dram_tensor("wk", [D, DH], F16, kind="ExternalInput").ap()
    wv_d = nc.dram_tensor("wv", [D, DH], F16, kind="ExternalInput").ap()
    wo_d = nc.dram_tensor("wo", [DH, D], F16, kind="ExternalInput").ap()
    bq_d = nc.dram_tensor("bq", [128, 2], F32, kind="ExternalInput").ap()
    sel_d = nc.dram_tensor("sel", [64, 128], F32R, kind="ExternalInput").ap()
    zr_d = nc.dram_tensor("zr", [64, 512], F32R, kind="ExternalInput").ap()
    y_d = nc.dram_tensor("y", [D, S], F16, kind="ExternalOutput").ap()

    with tile.TileContext(nc) as tc, ExitStack() as ctx:
        sbw = ctx.enter_context(tc.tile_pool(name="sbw", bufs=1))
        sbx = ctx.enter_context(tc.tile_pool(name="sbx", bufs=1))
        sbd = ctx.enter_context(tc.tile_pool(name="sbd", bufs=1))
        sbe = ctx.enter_context(tc.tile_pool(name="sbe", bufs=1))
        sbo = ctx.enter_context(tc.tile_pool(name="sbo", bufs=1))
        ps = ctx.enter_context(tc.tile_pool(name="ps", bufs=1, space="PSUM"))

        # ---- static tiles --------------------------------------------------
        wq_t = sbw.tile([128, KCD * DH], F16)
        wk_t = sbw.tile([128, KCD * DH], F16)
        wv_t = sbw.tile([128, KCD * DH], F16)
        wo_t = sbw.tile([128, 2 * D], F16)
        bq_t = sbw.tile([128, 2], F32)
        sel_t = sbw.tile([64, 128], F32R)
        recip = sbw.tile([64, 512], F32R)
        scr = sbw.tile([1, 2], F32)

        # x inputs as (kc, cb) pieces: [128, 512] each
        xq_t = [[sbx.tile([128, 512], F16, name=f"xq{kc}_{cb}")
                 for cb in range(4)] for kc in range(KCD)]
        xk_t = [[sbx.tile([128, 512], F16, name=f"xk{kc}_{cb}")
                 for cb in range(4)] for kc in range(KCD)]
        xv_t = [[sbx.tile([128, 512], F16, name=f"xv{kc}_{cb}")
                 for cb in range(4)] for kc in range(KCD)]

        # projection outputs
        qt_t = [sbd.tile([128, S], F16, name=f"qt{p}") for p in range(2)]
        kt_t = [sbd.tile([128, S], F16, name=f"kt{p}") for p in range(2)]
        # V with ones column per (chunk, head): per tb 4 x 65 cols
        v_t = sbd.tile([128, NKC * 4 * 65], F16)
        outsc = [sbd.tile([128, S], F16, name=f"outsc{p}") for p in range(2)]

        # ---- DMA issue (consumption order, two queues) ---------------------
        q1 = nc.sync.dma_start
        q2 = nc.scalar.dma_start

        def dma_w(dst, src_d, q):
            for kc in range(KCD):
                q(dst[:, kc * DH:(kc + 1) * DH], src_d[kc * 128:(kc + 1) * 128, :])

        def dma_x(t, src_d, cb, q):
            for kc in range(KCD):
                q(t[kc][cb][:], src_d[kc * 128:(kc + 1) * 128, cb * 512:(cb + 1) * 512])

        q2(bq_t[:], bq_d)
        q2(sel_t[:], sel_d)
        q2(recip[:], zr_d)
        dma_w(wk_t, wk_d, q1)
        dma_x(xk_t, xk_d, 0, q1)
        dma_w(wq_t, wq_d, q2)
        dma_x(xq_t, xq_d, 0, q2)
        dma_w(wv_t, wv_d, q1)
        dma_x(xv_t, xv_d, 0, q1)
        for cb in range(1, 4):
            dma_x(xk_t, xk_d, cb, q1)
            dma_x(xv_t, xv_d, cb, q1)
            dma_x(xq_t, xq_d, cb, q1)
        for kc2 in range(2):
            q1(wo_t[:, kc2 * D:(kc2 + 1) * D], wo_d[kc2 * 128:(kc2 + 1) * 128, :])

        # exp table pre-load while DMAs stream (first ACTIVATE triggers it)
        nc.scalar.activation(scr[0:1, 0:1], bq_t[0:1, 0:1], Exp)
        # fill v_t with ones; vproj copies overwrite the 64 value columns of
        # each 65-wide head block, leaving col 64 = 1.0 (the rowsum column)
        nc.vector.memset(v_t[:], 1.0)

        # ---- filler generators (one slot of work per yield) ----------------
        def kproj_gen(kb, pb):
            acc = ps.tile([128, 512], F32, name="kacc", tag="pacc", bufs=2)
            for kc in range(KCD):
                nc.tensor.matmul(
                    acc[:],
                    wk_t[:, kc * DH + pb * 128:kc * DH + (pb + 1) * 128],
                    xk_t[kc][kb][:],
                    start=(kc == 0), stop=(kc == KCD - 1))
                if kc == 3:
                    yield
            with nc.allow_low_precision(reason="fp16 scores"):
                nc.vector.tensor_copy(kt_t[pb][:, kb * 512:(kb + 1) * 512], acc[:])
            yield

        def qproj_gen(qb, pb):
            acc = ps.tile([128, 512], F32, name="qacc", tag="pacc", bufs=2)
            for kc in range(KCD):
                nc.tensor.matmul(
                    acc[:],
                    wq_t[:, kc * DH + pb * 128:kc * DH + (pb + 1) * 128],
                    xq_t[kc][qb][:],
                    start=(kc == 0), stop=(kc == KCD - 1))
                if kc == 3:
                    yield
            with nc.allow_low_precision(reason="fp16 scores"):
                nc.vector.tensor_scalar_add(qt_t[pb][:, qb * 512:(qb + 1) * 512],
                                            acc[:], bq_t[:, pb:pb + 1])
            yield

        def vproj_gen(pair, cb):
            # v_t head-pair half for key chunks tb = 4*cb .. 4*cb+3
            for tb in range(4 * cb, 4 * cb + 4):
                acc = ps.tile([128, 128], F32, name="vacc", tag="pacc", bufs=2)
                for kc in range(KCD):
                    nc.tensor.matmul(
                        acc[:],
                        xv_t[kc][cb][:, (tb - 4 * cb) * 128:(tb - 4 * cb + 1) * 128],
                        wv_t[:, kc * DH + pair * 128:kc * DH + pair * 128 + 128],
                        start=(kc == 0), stop=(kc == KCD - 1))
                    if kc == 3:
                        yield
                base = tb * 260 + pair * 130
                with nc.allow_low_precision(reason="fp16 attn weights"):
                    nc.vector.tensor_copy(v_t[:, base:base + 64], acc[:, 0:64])
                    nc.vector.tensor_copy(v_t[:, base + 65:base + 129], acc[:, 64:128])
                yield

        def norm_gen(pair, qb, outA, outB):
            # softmax denominators sit in row 64 of outA/outB
            with nc.allow_low_precision(reason="f32r denominators"):
                nc.vector.tensor_copy(recip[0:1, :], outA[64:65, :])
                nc.vector.tensor_copy(recip[32:33, :], outB[64:65, :])
            yield
            bc_ps = ps.tile([128, 512], F32, name="bc", tag="pacc", bufs=2)
            nc.tensor.matmul(bc_ps[:], sel_t[:], recip[:], start=True, stop=True)
            bc_sb = sbo.tile([128, 512], F32, name="bc_sb", tag="bcastr", bufs=2)
            nc.vector.reciprocal_approx_fast(bc_sb[:], bc_ps[:])
            yield
            with nc.allow_low_precision(reason="fp16 out"):
                nc.vector.tensor_mul(outsc[pair][0:64, qb * 512:(qb + 1) * 512],
                                     outA[0:64, :], bc_sb[0:64, :])
            yield
            with nc.allow_low_precision(reason="fp16 out"):
                nc.vector.tensor_mul(outsc[pair][64:128, qb * 512:(qb + 1) * 512],
                                     outB[0:64, :], bc_sb[64:128, :])
            yield
            if pair == 1:
                for ypb in range(D // 128):
                    yacc = ps.tile([128, 512], F32, name="yacc", tag="pacc", bufs=2)
                    for kc2 in range(2):
                        nc.tensor.matmul(
                            yacc[:],
                            wo_t[:, kc2 * D + ypb * 128:kc2 * D + (ypb + 1) * 128],
                            outsc[kc2][:, qb * 512:(qb + 1) * 512],
                            start=(kc2 == 0), stop=(kc2 == 1))
                    ysb = sbo.tile([128, 512], F16, name="ysb", tag="ysb", bufs=3)
                    with nc.allow_low_precision(reason="fp16 partial y"):
                        nc.vector.tensor_copy(ysb[:], yacc[:])
                    nc.sync.dma_start(
                        y_d[ypb * 128:(ypb + 1) * 128, qb * 512:(qb + 1) * 512],
                        ysb[:])
                    yield

        def drain(gen):
            if gen is not None:
                for _ in gen:
                    pass

        def step(gen):
            try:
                next(gen)
                return gen
            except StopIteration:
                return None

        # ---- filler scheduling --------------------------------------------
        # Projection work is emitted opportunistically (one step per chunk
        # slot, in the priority order below), but each consumer force-drains
        # its producer first, so correctness never depends on the pacing.
        gens = {}
        for kb in range(4):
            for pb in range(2):
                gens[("k", kb, pb)] = kproj_gen(kb, pb)
                gens[("q", kb, pb)] = qproj_gen(kb, pb)
        for pb in range(2):
            for cb in range(4):
                gens[("v", pb, cb)] = vproj_gen(pb, cb)

        order = [("k", 0, 0), ("q", 0, 0), ("v", 0, 0),
                 ("k", 1, 0), ("v", 0, 1), ("k", 2, 0), ("v", 0, 2),
                 ("k", 3, 0), ("v", 0, 3), ("q", 1, 0), ("q", 2, 0),
                 ("k", 0, 1), ("v", 1, 0), ("k", 1, 1), ("q", 3, 0),
                 ("k", 2, 1), ("v", 1, 1), ("k", 3, 1), ("q", 0, 1),
                 ("v", 1, 2), ("v", 1, 3), ("q", 1, 1), ("q", 2, 1),
                 ("q", 3, 1)]
        fillers = list(order)

        def ensure(key):
            if key in gens:
                drain(gens.pop(key))
                if key in fillers:
                    fillers.remove(key)

        def fill_slot():
            while fillers:
                key = fillers[0]
                g = gens.get(key)
                if g is None:
                    fillers.pop(0)
                    continue
                try:
                    next(g)
                    return
                except StopIteration:
                    gens.pop(key, None)
                    fillers.pop(0)

        norm_g = None
        for pair in range(2):
            ktp, qtp = kt_t[pair], qt_t[pair]
            for qb in range(NQB):
                outA = ps.tile([65, 512], F32, name="outA", tag="outA", bufs=1)
                outB = ps.tile([65, 512], F32, name="outB", tag="outB", bufs=1)
                vbase = pair * 130
                ets = {}

                def attn_v(kc):
                    ensure(("v", pair, kc // 4))
                    et = ets.pop(kc)
                    nc.tensor.matmul(outA[:],
                                     v_t[:, kc * 260 + vbase:kc * 260 + vbase + 65],
                                     et[:, 0:512],
                                     start=(kc == 0), stop=(kc == NKC - 1))
                    nc.tensor.matmul(outB[:],
                                     v_t[:, kc * 260 + vbase + 65:kc * 260 + vbase + 130],
                                     et[:, 512:1024],
                                     start=(kc == 0), stop=(kc == NKC - 1))

                ensure(("q", qb, pair))
                for kc in range(NKC):
                    ensure(("k", kc // 4, pair))
                    sc = ps.tile([128, 1024], F32, name="sc", tag="sc", bufs=2)
                    nc.tensor.matmul(sc[:, 0:512],
                                     ktp[0:64, kc * 128:(kc + 1) * 128],
                                     qtp[0:64, qb * 512:(qb + 1) * 512],
                                     start=True, stop=True)
                    nc.tensor.matmul(sc[:, 512:1024],
                                     ktp[64:128, kc * 128:(kc + 1) * 128],
                                     qtp[64:128, qb * 512:(qb + 1) * 512],
                                     start=True, stop=True)
                    et = sbe.tile([128, 1024], F16, name="et", tag="et", bufs=4)
                    ets[kc] = et
                    with nc.allow_low_precision(reason="fp16 attn weights"):
                        nc.scalar.activation(et[:], sc[:], Exp, scale=0.125)
                    # norm of the previous block first (frees outA/outB),
                    # then one step of projection/output filler work
                    if norm_g is not None:
                        norm_g = step(norm_g)
                    else:
                        fill_slot()
                    if kc >= LAG:
                        attn_v(kc - LAG)
                for kc in range(NKC - LAG, NKC):
                    attn_v(kc)
                drain(norm_g)
                norm_g = norm_gen(pair, qb, outA, outB)

        drain(norm_g)
        for g in list(gens.values()):
            drain(g)

    nc.compile()
    return nc


def _get_nc():
    if "nc" not in _CACHE:
        _CACHE["nc"] = _build_nc()
    return _CACHE["nc"]


def kernel(q, k, v, Wq, bq, Wk, bk, Wv, bv, Wo, bo, _trace=False, _tmpdir=None):
    from concourse.bass_utils import run_bass_kernel_spmd

    q = np.asarray(q, np.float32)
    k = np.asarray(k, np.float32)
    v = np.asarray(v, np.float32)
    Wq = np.asarray(Wq, np.float32)
    Wk = np.asarray(Wk, np.float32)
    Wv = np.asarray(Wv, np.float32)
    Wo = np.asarray(Wo, np.float32)
    bq = np.asarray(bq, np.float32)
    bv = np.asarray(bv, np.float32)
    bo = np.asarray(bo, np.float32)

    nc = _get_nc()

    sel = np.zeros((64, 128), np.float32)
    sel[0, 0:64] = 1.0
    sel[32, 64:128] = 1.0

    xT = {}
    for b in range(B):
        xT[("q", b)] = np.ascontiguousarray(q[b].T).astype(np.float16)
        xT[("k", b)] = np.ascontiguousarray(k[b].T).astype(np.float16)
        xT[("v", b)] = np.ascontiguousarray(v[b].T).astype(np.float16)

    in_maps = []
    for c in range(8):
        b, g = c // G, c % G
        gr = slice(g * DH, (g + 1) * DH)
        in_maps.append({
            "xq": xT[("q", b)],
            "xk": xT[("k", b)],
            "xv": xT[("v", b)],
            "wq": np.ascontiguousarray(Wq[gr, :].T).astype(np.float16),
            "wk": np.ascontiguousarray(Wk[gr, :].T).astype(np.float16),
            "wv": np.ascontiguousarray(Wv[gr, :].T).astype(np.float16),
            "wo": np.ascontiguousarray(Wo[:, gr].T).astype(np.float16),
            "bq": np.ascontiguousarray(bq[gr].reshape(2, 128).T),
            "sel": sel,
            "zr": np.zeros((64, 512), np.float32),
        })

    kwargs = {}
    if _trace:
        kwargs = dict(trace=True, tmpdir=_tmpdir)
    res = run_bass_kernel_spmd(nc, in_maps, core_ids=list(range(8)), **kwargs)

    # host reduce: y[b] = sum_g y_g^T.T  (+ bias terms folded host-side)
    bias_row = bv @ Wo.T + bo                     # [D]
    out = np.empty((B, S, D), np.float32)
    for b in range(B):
        acc = np.zeros((S, D), np.float32)
        for g in range(G):
            acc += res.results[b * G + g]["y"].T.astype(np.float32)
        out[b] = acc + bias_row[None, :]
    if _trace:
        out = (out, res)
    return out


# revision 26
# speedup vs baseline: 1.4298x; 1.2503x over previous
"""Multi-head attention (B=2, S=2048, D=1024, H=16) on 8 NeuronCores.

Sharding: batch x head-group (2 batches x 4 groups of 4 heads). Each core:
  - projects its group's Q^T/K^T (fp16, [256, 2048]) and V (fp16, [2048, 256])
  - attention per head-pair: scores^T via row-tiled fp16 matmuls, exp on
    ScalarE (fp16 out), attn@V with a ones-column fused into the V stationary
    so PSUM row 64 accumulates the softmax denominator for free
  - normalization via f32r selector-broadcast matmul + fast reciprocal
  - partial output projection y_g^T = Wo[:, g] @ out_g^T (fp16 in/out)
Host: y[b] = sum_g y_g^T.T + bv @ Wo.T + bo.  K-bias drops out of softmax;
Q-bias applied on device; V-bias folded host-side.

Scheduling: the attention inner loop is ScalarE(exp)-bound (~1.15us per
128-key chunk-slot).  All other PE work (K/V/Q projections for later blocks,
output projection, normalization) is fed from a persistent filler queue, one
step per chunk slot, so it hides in the exp slack instead of stalling ScalarE
between query blocks.  Input DMAs are column-block pieces on two queues in
consumption order so compute starts ~8us in.
"""
import numpy as np

B = 2
S = 2048
D = 1024
H = 16
DK = 64
G = 4              # head-groups (cores per batch)
HG = H // G        # heads per group = 4
DH = HG * DK       # group dims = 256
NQB = S // 512     # query blocks
NKC = S // 128     # key chunks
KCD = D // 128     # d_model chunks
LAG = 3            # attn@V trails scores/exp by this many chunk slots

_CACHE = {}


def _build_nc():
    import concourse.tile as tile
    import concourse.bacc as bacc
    from concourse import mybir
    from contextlib import ExitStack

    F32 = mybir.dt.float32
    F32R = mybir.dt.float32r
    F16 = mybir.dt.float16
    Exp = mybir.ActivationFunctionType.Exp

    nc = bacc.Bacc("TRN2", target_bir_lowering=False, debug=False)

    xq_d = nc.dram_tensor("xq", [D, S], F16, kind="ExternalInput").ap()
    xk_d = nc.dram_tensor("xk", [D, S], F16, kind="ExternalInput").ap()
    xv_d = nc.dram_tensor("xv", [D, S], F16, kind="ExternalInput").ap()
    wq_d = nc.dram_tensor("wq", [D, DH], F16, kind="ExternalInput").ap()
    wk_d = nc.dram_tensor("wk", [D, DH], F16, kind="ExternalInput").ap()
    wv_d = nc.dram_tensor("wv", [D, DH], F16, kind="ExternalInput").ap()
    wo_d = nc.dram_tensor("wo", [DH, D], F16, kind="ExternalInput").ap()
    bq_d = nc.dram_tensor("bq", [128, 2], F32, kind="ExternalInput").ap()
    sel_d = nc.dram_tensor("sel", [64, 128], F32R, kind="ExternalInput").ap()
    zr_d = nc.dram_tensor("zr", [64, 512], F32R, kind="ExternalInput").ap()
    y_d = nc.dram_tensor("y", [D, S], F16, kind="ExternalOutput").ap()

    with tile.TileContext(nc) as tc, ExitStack() as ctx:
        sbw = ctx.enter_context(tc.tile_pool(name="sbw", bufs=1))
        sbx = ctx.enter_context(tc.tile_pool(name="sbx", bufs=1))
        sbd = ctx.enter_context(tc.tile_pool(name="sbd", bufs=1))
        sbe = ctx.enter_context(tc.tile_pool(name="sbe", bufs=1))
        sbo = ctx.enter_context(tc.tile_pool(name="sbo", bufs=1))
        ps = ctx.enter_context(tc.tile_pool(name="ps", bufs=1, space="PSUM"))

        # ---- static tiles --------------------------------------------------
        wq_t = sbw.tile([128, KCD * DH], F16)
        wk_t = sbw.tile([128, KCD * DH], F16)
        wv_t = sbw.tile([128, KCD * DH], F16)
        wo_t = sbw.tile([128, 2 * D], F16)
        bq_t = sbw.tile([128, 2], F32)
        sel_t = sbw.tile([64, 128], F32R)
        recip = sbw.tile([64, 512], F32R)
        scr = sbw.tile([1, 2], F32)

        # x inputs as column-block tiles [128, 8*512]: chunk kc of block cb
        # lives at cols [kc*512, (kc+1)*512) of xX_t[cb]
        xq_t = [sbx.tile([128, KCD * 512], F16, name=f"xq{cb}") for cb in range(4)]
        xk_t = [sbx.tile([128, KCD * 512], F16, name=f"xk{cb}") for cb in range(4)]
        xv_t = [sbx.tile([128, KCD * 512], F16, name=f"xv{cb}") for cb in range(4)]

        # projection outputs
        qt_t = [sbd.tile([128, S], F16, name=f"qt{p}") for p in range(2)]
        kt_t = [sbd.tile([128, S], F16, name=f"kt{p}") for p in range(2)]
        # V with ones column per (chunk, head): per tb 4 x 65 cols
        v_t = sbd.tile([128, NKC * 4 * 65], F16)
        outsc = [sbd.tile([128, S], F16, name=f"outsc{p}") for p in range(2)]

        # ---- DMA issue (consumption order, two queues) ---------------------
        q1 = nc.sync.dma_start
        q2 = nc.scalar.dma_start

        # one descriptor per (tensor, column-block): 3D AP folds the 8 d_model
        # chunks into one transfer (desc processing cost is per-instruction)
        def dma_w(dst, src_d, q):
            q(dst[:].rearrange("p (k c) -> p k c", k=KCD),
              src_d[:, :].rearrange("(k p) c -> p k c", p=128))

        def dma_x(t, src_d, cb, q):
            q(t[cb][:].rearrange("p (k c) -> p k c", k=KCD),
              src_d[:, cb * 512:(cb + 1) * 512].rearrange("(k p) c -> p k c", p=128))

        q2(bq_t[:], bq_d)
        q2(sel_t[:], sel_d)
        q2(recip[:], zr_d)
        dma_w(wk_t, wk_d, q1)
        dma_x(xk_t, xk_d, 0, q1)
        dma_w(wq_t, wq_d, q2)
        dma_x(xq_t, xq_d, 0, q2)
        dma_w(wv_t, wv_d, q1)
        dma_x(xv_t, xv_d, 0, q1)
        for cb in range(1, 4):
            dma_x(xk_t, xk_d, cb, q1)
            dma_x(xv_t, xv_d, cb, q1)
            dma_x(xq_t, xq_d, cb, q2)
        q2(wo_t[:].rearrange("p (k c) -> p k c", k=2),
           wo_d[:, :].rearrange("(k p) c -> p k c", p=128))

        # exp table pre-load while DMAs stream (first ACTIVATE triggers it)
        nc.scalar.activation(scr[0:1, 0:1], bq_t[0:1, 0:1], Exp)
        # fill v_t with ones; vproj copies overwrite the 64 value columns of
        # each 65-wide head block, leaving col 64 = 1.0 (the rowsum column)
        nc.vector.memset(v_t[:], 1.0)

        # ---- filler generators (one slot of work per yield) ----------------
        def kproj_gen(kb, pb):
            acc = ps.tile([128, 512], F32, name="kacc", tag="pacc", bufs=2)
            for kc in range(KCD):
                nc.tensor.matmul(
                    acc[:],
                    wk_t[:, kc * DH + pb * 128:kc * DH + (pb + 1) * 128],
                    xk_t[kb][:, kc * 512:(kc + 1) * 512],
                    start=(kc == 0), stop=(kc == KCD - 1))
                if kc == 3:
                    yield
            with nc.allow_low_precision(reason="fp16 scores"):
                nc.vector.tensor_copy(kt_t[pb][:, kb * 512:(kb + 1) * 512], acc[:])
            yield

        def qproj_gen(qb, pb):
            acc = ps.tile([128, 512], F32, name="qacc", tag="pacc", bufs=2)
            for kc in range(KCD):
                nc.tensor.matmul(
                    acc[:],
                    wq_t[:, kc * DH + pb * 128:kc * DH + (pb + 1) * 128],
                    xq_t[qb][:, kc * 512:(kc + 1) * 512],
                    start=(kc == 0), stop=(kc == KCD - 1))
                if kc == 3:
                    yield
            with nc.allow_low_precision(reason="fp16 scores"):
                nc.vector.tensor_scalar_add(qt_t[pb][:, qb * 512:(qb + 1) * 512],
                                            acc[:], bq_t[:, pb:pb + 1])
            yield

        def vproj_gen(cb):
            # V (all 4 heads) for key chunks tb = 4*cb .. 4*cb+3
            for tb in range(4 * cb, 4 * cb + 4):
                acc = ps.tile([128, DH], F32, name="vacc", tag="pacc", bufs=2)
                for kc in range(KCD):
                    nc.tensor.matmul(
                        acc[:],
                        xv_t[cb][:, kc * 512 + (tb - 4 * cb) * 128:
                                 kc * 512 + (tb - 4 * cb + 1) * 128],
                        wv_t[:, kc * DH:(kc + 1) * DH],
                        start=(kc == 0), stop=(kc == KCD - 1))
                    if kc == 3:
                        yield
                with nc.allow_low_precision(reason="fp16 attn weights"):
                    for h in range(4):
                        nc.vector.tensor_copy(
                            v_t[:, tb * 260 + h * 65:tb * 260 + h * 65 + 64],
                            acc[:, h * 64:(h + 1) * 64])
                yield

        def norm_gen(pair, qb, outA, outB):
            # softmax denominators sit in row 64 of outA/outB
            with nc.allow_low_precision(reason="f32r denominators"):
                nc.vector.tensor_copy(recip[0:1, :], outA[64:65, :])
                nc.vector.tensor_copy(recip[32:33, :], outB[64:65, :])
            yield
            bc_ps = ps.tile([128, 512], F32, name="bc", tag="pacc", bufs=2)
            nc.tensor.matmul(bc_ps[:], sel_t[:], recip[:], start=True, stop=True)
            bc_sb = sbo.tile([128, 512], F32, name="bc_sb", tag="bcastr", bufs=2)
            nc.vector.reciprocal_approx_fast(bc_sb[:], bc_ps[:])
            yield
            with nc.allow_low_precision(reason="fp16 out"):
                nc.vector.tensor_mul(outsc[pair][0:64, qb * 512:(qb + 1) * 512],
                                     outA[0:64, :], bc_sb[0:64, :])
            yield
            with nc.allow_low_precision(reason="fp16 out"):
                nc.vector.tensor_mul(outsc[pair][64:128, qb * 512:(qb + 1) * 512],
                                     outB[0:64, :], bc_sb[64:128, :])
            yield
            if pair == 1:
                for ypb in range(D // 128):
                    yacc = ps.tile([128, 512], F32, name="yacc", tag="pacc", bufs=2)
                    for kc2 in range(2):
                        nc.tensor.matmul(
                            yacc[:],
                            wo_t[:, kc2 * D + ypb * 128:kc2 * D + (ypb + 1) * 128],
                            outsc[kc2][:, qb * 512:(qb + 1) * 512],
                            start=(kc2 == 0), stop=(kc2 == 1))
                    ysb = sbo.tile([128, 512], F16, name="ysb", tag="ysb", bufs=3)
                    with nc.allow_low_precision(reason="fp16 partial y"):
                        nc.vector.tensor_copy(ysb[:], yacc[:])
                    nc.scalar.dma_start(
                        y_d[ypb * 128:(ypb + 1) * 128, qb * 512:(qb + 1) * 512],
                        ysb[:])
                    yield

        def drain(gen):
            if gen is not None:
                for _ in gen:
                    pass

        def step(gen):
            try:
                next(gen)
                return gen
            except StopIteration:
                return None

        # ---- filler scheduling --------------------------------------------
        # Projection work is emitted opportunistically (one step per chunk
        # slot, in the priority order below), but each consumer force-drains
        # its producer first, so correctness never depends on the pacing.
        gens = {}
        for kb in range(4):
            for pb in range(2):
                gens[("k", kb, pb)] = kproj_gen(kb, pb)
                gens[("q", kb, pb)] = qproj_gen(kb, pb)
        for cb in range(4):
            gens[("v", cb)] = vproj_gen(cb)

        order = [("k", 0, 0), ("q", 0, 0), ("v", 0),
                 ("k", 1, 0), ("v", 1), ("k", 2, 0), ("v", 2),
                 ("k", 3, 0), ("v", 3), ("q", 1, 0), ("q", 2, 0),
                 ("k", 0, 1), ("k", 1, 1), ("q", 3, 0),
                 ("k", 2, 1), ("k", 3, 1), ("q", 0, 1),
                 ("q", 1, 1), ("q", 2, 1), ("q", 3, 1)]
        fillers = list(order)

        def ensure(key):
            if key in gens:
                drain(gens.pop(key))
                if key in fillers:
                    fillers.remove(key)

        def fill_slot():
            while fillers:
                key = fillers[0]
                g = gens.get(key)
                if g is None:
                    fillers.pop(0)
                    continue
                try:
                    next(g)
                    return
                except StopIteration:
                    gens.pop(key, None)
                    fillers.pop(0)

        norm_g = None
        for pair in range(2):
            ktp, qtp = kt_t[pair], qt_t[pair]
            for qb in range(NQB):
                outA = ps.tile([65, 512], F32, name="outA", tag="outA", bufs=1)
                outB = ps.tile([65, 512], F32, name="outB", tag="outB", bufs=1)
                vbase = pair * 130
                ets = {}

                def attn_v(kc):
                    ensure(("v", kc // 4))
                    et = ets.pop(kc)
                    nc.tensor.matmul(outA[:],
                                     v_t[:, kc * 260 + vbase:kc * 260 + vbase + 65],
                                     et[:, 0:512],
                                     start=(kc == 0), stop=(kc == NKC - 1))
                    nc.tensor.matmul(outB[:],
                                     v_t[:, kc * 260 + vbase + 65:kc * 260 + vbase + 130],
                                     et[:, 512:1024],
                                     start=(kc == 0), stop=(kc == NKC - 1))

                ensure(("q", qb, pair))
                for kc in range(NKC):
                    ensure(("k", kc // 4, pair))
                    sc = ps.tile([128, 1024], F32, name="sc", tag="sc", bufs=2)
                    nc.tensor.matmul(sc[:, 0:512],
                                     ktp[0:64, kc * 128:(kc + 1) * 128],
                                     qtp[0:64, qb * 512:(qb + 1) * 512],
                                     start=True, stop=True)
                    nc.tensor.matmul(sc[:, 512:1024],
                                     ktp[64:128, kc * 128:(kc + 1) * 128],
                                     qtp[64:128, qb * 512:(qb + 1) * 512],
                                     start=True, stop=True)
                    et = sbe.tile([128, 1024], F16, name="et", tag="et", bufs=4)
                    ets[kc] = et
                    with nc.allow_low_precision(reason="fp16 attn weights"):
                        nc.scalar.activation(et[:], sc[:], Exp, scale=0.125)
                    # norm of the previous block first (frees outA/outB),
                    # then one step of projection/output filler work
                    if norm_g is not None:
                        norm_g = step(norm_g)
                    else:
                        fill_slot()
                    if kc >= LAG:
                        attn_v(kc - LAG)
                for kc in range(NKC - LAG, NKC):
                    attn_v(kc)
                drain(norm_g)
                norm_g = norm_gen(pair, qb, outA, outB)

        drain(norm_g)
        for g in list(gens.values()):
            drain(g)

    nc.compile()
    return nc


def _get_nc():
    if "nc" not in _CACHE:
        _CACHE["nc"] = _build_nc()
    return _CACHE["nc"]


def kernel(q, k, v, Wq, bq, Wk, bk, Wv, bv, Wo, bo, _trace=False, _tmpdir=None):
    from concourse.bass_utils import run_bass_kernel_spmd

    q = np.asarray(q, np.float32)
    k = np.asarray(k, np.float32)
    v = np.asarray(v, np.float32)
    Wq = np.asarray(Wq, np.float32)
    Wk = np.asarray(Wk, np.float32)
    Wv = np.asarray(Wv, np.float32)
    Wo = np.asarray(Wo, np.float32)
    bq = np.asarray(bq, np.float32)
    bv = np.asarray(bv, np.float32)
    bo = np.asarray(bo, np.float32)

    nc = _get_nc()

    sel = np.zeros((64, 128), np.float32)
    sel[0, 0:64] = 1.0
    sel[32, 64:128] = 1.0

    xT = {}
    for b in range(B):
        xT[("q", b)] = np.ascontiguousarray(q[b].T).astype(np.float16)
        xT[("k", b)] = np.ascontiguousarray(k[b].T).astype(np.float16)
        xT[("v", b)] = np.ascontiguousarray(v[b].T).astype(np.float16)

    in_maps = []
    for c in range(8):
        b, g = c // G, c % G
        gr = slice(g * DH, (g + 1) * DH)
        in_maps.append({
            "xq": xT[("q", b)],
            "xk": xT[("k", b)],
            "xv": xT[("v", b)],
            "wq": np.ascontiguousarray(Wq[gr, :].T).astype(np.float16),
            "wk": np.ascontiguousarray(Wk[gr, :].T).astype(np.float16),
            "wv": np.ascontiguousarray(Wv[gr, :].T).astype(np.float16),
            "wo": np.ascontiguousarray(Wo[:, gr].T).astype(np.float16),
            "bq": np.ascontiguousarray(bq[gr].reshape(2, 128).T),
            "sel": sel,
            "zr": np.zeros((64, 512), np.float32),
        })

    kwargs = {}
    if _trace:
        kwargs = dict(trace=True, tmpdir=_tmpdir)
    res = run_bass_kernel_spmd(nc, in_maps, core_ids=list(range(8)), **kwargs)

    # host reduce: y[b] = sum_g y_g^T.T  (+ bias terms folded host-side)
    bias_row = bv @ Wo.T + bo                     # [D]
    out = np.empty((B, S, D), np.float32)
    for b in range(B):
        acc = np.zeros((S, D), np.float32)
        for g in range(G):
            acc += res.results[b * G + g]["y"].T.astype(np.float32)
        out[b] = acc + bias_row[None, :]
    if _trace:
        out = (out, res)
    return out


# revision 32
# speedup vs baseline: 1.4673x; 1.0262x over previous
"""Multi-head attention (B=2, S=2048, D=1024, H=16) on 8 NeuronCores.

Sharding: batch x head-group (2 batches x 4 groups of 4 heads). Each core:
  - projects its group's Q^T/K^T (fp16, [256, 2048]) and V (fp16, [2048, 256])
  - attention per head-pair: scores^T via row-tiled fp16 matmuls, exp on
    ScalarE (fp16 out), attn@V with a ones-column fused into the V stationary
    so PSUM row 64 accumulates the softmax denominator for free
  - normalization via f32r selector-broadcast matmul + fast reciprocal
  - partial output projection y_g^T = Wo[:, g] @ out_g^T (fp16 in/out)
Host: y[b] = sum_g y_g^T.T + bv @ Wo.T + bo.  K-bias drops out of softmax;
Q-bias applied on device; V-bias folded host-side.

Scheduling: the attention inner loop is ScalarE(exp)-bound (~1.15us per
128-key chunk-slot).  All other PE work (K/V/Q projections for later blocks,
output projection, normalization) is fed from a persistent filler queue, one
step per chunk slot, so it hides in the exp slack instead of stalling ScalarE
between query blocks.  Input DMAs are column-block pieces on two queues in
consumption order so compute starts ~8us in.
"""
import numpy as np

B = 2
S = 2048
D = 1024
H = 16
DK = 64
G = 4              # head-groups (cores per batch)
HG = H // G        # heads per group = 4
DH = HG * DK       # group dims = 256
NQB = S // 512     # query blocks
NKC = S // 128     # key chunks
KCD = D // 128     # d_model chunks
LAG = 3            # attn@V trails scores/exp by this many chunk slots

_CACHE = {}


def _build_nc():
    import concourse.tile as tile
    import concourse.bacc as bacc
    from concourse import mybir
    from contextlib import ExitStack

    F32 = mybir.dt.float32
    F32R = mybir.dt.float32r
    F16 = mybir.dt.float16
    Exp = mybir.ActivationFunctionType.Exp

    nc = bacc.Bacc("TRN2", target_bir_lowering=False, debug=False)

    # x inputs host-reordered: row cb*128+p, col kc*512+c = x^T[kc*128+p,
    # cb*512+c], so each column-block cb is one [128 x 8KB-row] descriptor
    xq_d = nc.dram_tensor("xq", [512, KCD * 512], F16, kind="ExternalInput").ap()
    xk_d = nc.dram_tensor("xk", [512, KCD * 512], F16, kind="ExternalInput").ap()
    xv_d = nc.dram_tensor("xv", [512, KCD * 512], F16, kind="ExternalInput").ap()
    # weights host-reordered to [128, kc*DH+c] (chunk-major columns)
    wq_d = nc.dram_tensor("wq", [128, KCD * DH], F16, kind="ExternalInput").ap()
    wk_d = nc.dram_tensor("wk", [128, KCD * DH], F16, kind="ExternalInput").ap()
    wv_d = nc.dram_tensor("wv", [128, KCD * DH], F16, kind="ExternalInput").ap()
    wo_d = nc.dram_tensor("wo", [128, 2 * D], F16, kind="ExternalInput").ap()
    bq_d = nc.dram_tensor("bq", [128, 2], F32, kind="ExternalInput").ap()
    sel_d = nc.dram_tensor("sel", [64, 128], F32R, kind="ExternalInput").ap()
    zr_d = nc.dram_tensor("zr", [64, 512], F32R, kind="ExternalInput").ap()
    y_d = nc.dram_tensor("y", [D, S], F16, kind="ExternalOutput").ap()

    with tile.TileContext(nc) as tc, ExitStack() as ctx:
        sbw = ctx.enter_context(tc.tile_pool(name="sbw", bufs=1))
        sbx = ctx.enter_context(tc.tile_pool(name="sbx", bufs=1))
        sbd = ctx.enter_context(tc.tile_pool(name="sbd", bufs=1))
        sbe = ctx.enter_context(tc.tile_pool(name="sbe", bufs=1))
        sbo = ctx.enter_context(tc.tile_pool(name="sbo", bufs=1))
        ps = ctx.enter_context(tc.tile_pool(name="ps", bufs=1, space="PSUM"))

        # ---- static tiles --------------------------------------------------
        wq_t = sbw.tile([128, KCD * DH], F16)
        wk_t = sbw.tile([128, KCD * DH], F16)
        wv_t = sbw.tile([128, KCD * DH], F16)
        wo_t = sbw.tile([128, 2 * D], F16)
        bq_t = sbw.tile([128, 2], F32)
        sel_t = sbw.tile([64, 128], F32R)
        recip = sbw.tile([64, 512], F32R)
        scr = sbw.tile([1, 2], F32)

        # x inputs as column-block tiles [128, 8*512]: chunk kc of block cb
        # lives at cols [kc*512, (kc+1)*512) of xX_t[cb]
        xq_t = [sbx.tile([128, KCD * 512], F16, name=f"xq{cb}") for cb in range(4)]
        xk_t = [sbx.tile([128, KCD * 512], F16, name=f"xk{cb}") for cb in range(4)]
        xv_t = [sbx.tile([128, KCD * 512], F16, name=f"xv{cb}") for cb in range(4)]

        # projection outputs
        qt_t = [sbd.tile([128, S], F16, name=f"qt{p}") for p in range(2)]
        kt_t = [sbd.tile([128, S], F16, name=f"kt{p}") for p in range(2)]
        # V with ones column per (chunk, head): per tb 4 x 65 cols
        v_t = sbd.tile([128, NKC * 4 * 65], F16)
        outsc = [sbd.tile([128, S], F16, name=f"outsc{p}") for p in range(2)]

        # ---- DMA issue (consumption order, two queues) ---------------------
        q1 = nc.sync.dma_start
        q2 = nc.scalar.dma_start

        # one [128 x 8KB-row] descriptor per (tensor, column-block)
        def dma_w(dst, src_d, q):
            q(dst[:], src_d[:, :])

        def dma_x(t, src_d, cb, q):
            q(t[cb][:], src_d[cb * 128:(cb + 1) * 128, :])

        q2(bq_t[:], bq_d)
        q2(sel_t[:], sel_d)
        q2(recip[:], zr_d)
        dma_w(wk_t, wk_d, q1)
        dma_x(xk_t, xk_d, 0, q1)
        dma_w(wq_t, wq_d, q2)
        dma_x(xq_t, xq_d, 0, q2)
        dma_w(wv_t, wv_d, q1)
        dma_x(xv_t, xv_d, 0, q1)
        for cb in range(1, 4):
            dma_x(xk_t, xk_d, cb, q1)
            dma_x(xv_t, xv_d, cb, q1)
            dma_x(xq_t, xq_d, cb, q2)
        q2(wo_t[:], wo_d[:, :])

        # exp table pre-load while DMAs stream (first ACTIVATE triggers it)
        nc.scalar.activation(scr[0:1, 0:1], bq_t[0:1, 0:1], Exp)
        # fill v_t with ones; vproj copies overwrite the 64 value columns of
        # each 65-wide head block, leaving col 64 = 1.0 (the rowsum column)
        nc.vector.memset(v_t[:], 1.0)

        # ---- filler generators (one slot of work per yield) ----------------
        def kproj_gen(kb, pb):
            acc = ps.tile([128, 512], F32, name="kacc", tag="pacc", bufs=2)
            for kc in range(KCD):
                nc.tensor.matmul(
                    acc[:],
                    wk_t[:, kc * DH + pb * 128:kc * DH + (pb + 1) * 128],
                    xk_t[kb][:, kc * 512:(kc + 1) * 512],
                    start=(kc == 0), stop=(kc == KCD - 1))
                if kc == 3:
                    yield
            with nc.allow_low_precision(reason="fp16 scores"):
                nc.vector.tensor_copy(kt_t[pb][:, kb * 512:(kb + 1) * 512], acc[:])
            yield

        def qproj_gen(qb, pb):
            acc = ps.tile([128, 512], F32, name="qacc", tag="pacc", bufs=2)
            for kc in range(KCD):
                nc.tensor.matmul(
                    acc[:],
                    wq_t[:, kc * DH + pb * 128:kc * DH + (pb + 1) * 128],
                    xq_t[qb][:, kc * 512:(kc + 1) * 512],
                    start=(kc == 0), stop=(kc == KCD - 1))
                if kc == 3:
                    yield
            with nc.allow_low_precision(reason="fp16 scores"):
                nc.vector.tensor_scalar_add(qt_t[pb][:, qb * 512:(qb + 1) * 512],
                                            acc[:], bq_t[:, pb:pb + 1])
            yield

        def vproj_gen(cb):
            # V (all 4 heads) for key chunks tb = 4*cb .. 4*cb+3
            for tb in range(4 * cb, 4 * cb + 4):
                acc = ps.tile([128, DH], F32, name="vacc", tag="pacc", bufs=2)
                for kc in range(KCD):
                    nc.tensor.matmul(
                        acc[:],
                        xv_t[cb][:, kc * 512 + (tb - 4 * cb) * 128:
                                 kc * 512 + (tb - 4 * cb + 1) * 128],
                        wv_t[:, kc * DH:(kc + 1) * DH],
                        start=(kc == 0), stop=(kc == KCD - 1))
                    if kc == 3:
                        yield
                with nc.allow_low_precision(reason="fp16 attn weights"):
                    for h in range(4):
                        nc.vector.tensor_copy(
                            v_t[:, tb * 260 + h * 65:tb * 260 + h * 65 + 64],
                            acc[:, h * 64:(h + 1) * 64])
                yield

        def norm_gen(pair, qb, outA, outB):
            # softmax denominators sit in row 64 of outA/outB
            with nc.allow_low_precision(reason="f32r denominators"):
                nc.vector.tensor_copy(recip[0:1, :], outA[64:65, :])
                nc.vector.tensor_copy(recip[32:33, :], outB[64:65, :])
            yield
            bc_ps = ps.tile([128, 512], F32, name="bc", tag="pacc", bufs=2)
            nc.tensor.matmul(bc_ps[:], sel_t[:], recip[:], start=True, stop=True)
            bc_sb = sbo.tile([128, 512], F32, name="bc_sb", tag="bcastr", bufs=2)
            nc.vector.reciprocal_approx_fast(bc_sb[:], bc_ps[:])
            yield
            with nc.allow_low_precision(reason="fp16 out"):
                nc.vector.tensor_mul(outsc[pair][0:64, qb * 512:(qb + 1) * 512],
                                     outA[0:64, :], bc_sb[0:64, :])
            yield
            with nc.allow_low_precision(reason="fp16 out"):
                nc.vector.tensor_mul(outsc[pair][64:128, qb * 512:(qb + 1) * 512],
                                     outB[0:64, :], bc_sb[64:128, :])
            yield
            if pair == 1:
                for ypb in range(D // 128):
                    yacc = ps.tile([128, 512], F32, name="yacc", tag="pacc", bufs=2)
                    for kc2 in range(2):
                        nc.tensor.matmul(
                            yacc[:],
                            wo_t[:, kc2 * D + ypb * 128:kc2 * D + (ypb + 1) * 128],
                            outsc[kc2][:, qb * 512:(qb + 1) * 512],
                            start=(kc2 == 0), stop=(kc2 == 1))
                    ysb = sbo.tile([128, 512], F16, name="ysb", tag="ysb", bufs=3)
                    with nc.allow_low_precision(reason="fp16 partial y"):
                        nc.vector.tensor_copy(ysb[:], yacc[:])
                    nc.sync.dma_start(
                        y_d[ypb * 128:(ypb + 1) * 128, qb * 512:(qb + 1) * 512],
                        ysb[:])
                    yield

        def drain(gen):
            if gen is not None:
                for _ in gen:
                    pass

        def step(gen):
            try:
                next(gen)
                return gen
            except StopIteration:
                return None

        # ---- filler scheduling --------------------------------------------
        # Projection work is emitted opportunistically (one step per chunk
        # slot, in the priority order below), but each consumer force-drains
        # its producer first, so correctness never depends on the pacing.
        gens = {}
        for kb in range(4):
            for pb in range(2):
                gens[("k", kb, pb)] = kproj_gen(kb, pb)
                gens[("q", kb, pb)] = qproj_gen(kb, pb)
        for cb in range(4):
            gens[("v", cb)] = vproj_gen(cb)

        order = [("k", 0, 0), ("q", 0, 0), ("v", 0),
                 ("k", 1, 0), ("v", 1), ("k", 2, 0), ("v", 2),
                 ("k", 3, 0), ("v", 3), ("q", 1, 0), ("q", 2, 0),
                 ("k", 0, 1), ("k", 1, 1), ("q", 3, 0),
                 ("k", 2, 1), ("k", 3, 1), ("q", 0, 1),
                 ("q", 1, 1), ("q", 2, 1), ("q", 3, 1)]
        fillers = list(order)

        def ensure(key):
            if key in gens:
                drain(gens.pop(key))
                if key in fillers:
                    fillers.remove(key)

        def fill_slot():
            while fillers:
                key = fillers[0]
                g = gens.get(key)
                if g is None:
                    fillers.pop(0)
                    continue
                try:
                    next(g)
                    return
                except StopIteration:
                    gens.pop(key, None)
                    fillers.pop(0)

        norm_g = None
        for pair in range(2):
            ktp, qtp = kt_t[pair], qt_t[pair]
            for qb in range(NQB):
                outA = ps.tile([65, 512], F32, name="outA", tag="outA", bufs=1)
                outB = ps.tile([65, 512], F32, name="outB", tag="outB", bufs=1)
                vbase = pair * 130
                ets = {}

                def attn_v(kc):
                    ensure(("v", kc // 4))
                    et = ets.pop(kc)
                    nc.tensor.matmul(outA[:],
                                     v_t[:, kc * 260 + vbase:kc * 260 + vbase + 65],
                                     et[:, 0:512],
                                     start=(kc == 0), stop=(kc == NKC - 1))
                    nc.tensor.matmul(outB[:],
                                     v_t[:, kc * 260 + vbase + 65:kc * 260 + vbase + 130],
                                     et[:, 512:1024],
                                     start=(kc == 0), stop=(kc == NKC - 1))

                ensure(("q", qb, pair))
                for kc in range(NKC):
                    ensure(("k", kc // 4, pair))
                    sc = ps.tile([128, 1024], F32, name="sc", tag="sc", bufs=2)
                    nc.tensor.matmul(sc[:, 0:512],
                                     ktp[0:64, kc * 128:(kc + 1) * 128],
                                     qtp[0:64, qb * 512:(qb + 1) * 512],
                                     start=True, stop=True)
                    nc.tensor.matmul(sc[:, 512:1024],
                                     ktp[64:128, kc * 128:(kc + 1) * 128],
                                     qtp[64:128, qb * 512:(qb + 1) * 512],
                                     start=True, stop=True)
                    et = sbe.tile([128, 1024], F16, name="et", tag="et", bufs=4)
                    ets[kc] = et
                    with nc.allow_low_precision(reason="fp16 attn weights"):
                        nc.scalar.activation(et[:], sc[:], Exp, scale=0.125)
                    # norm of the previous block first (frees outA/outB),
                    # then one step of projection/output filler work
                    if norm_g is not None:
                        norm_g = step(norm_g)
                    else:
                        fill_slot()
                    if kc >= LAG:
                        attn_v(kc - LAG)
                for kc in range(NKC - LAG, NKC):
                    attn_v(kc)
                drain(norm_g)
                norm_g = norm_gen(pair, qb, outA, outB)

        drain(norm_g)
        for g in list(gens.values()):
            drain(g)

    nc.compile()
    return nc


def _get_nc():
    if "nc" not in _CACHE:
        _CACHE["nc"] = _build_nc()
    return _CACHE["nc"]


def kernel(q, k, v, Wq, bq, Wk, bk, Wv, bv, Wo, bo, _trace=False, _tmpdir=None):
    from concourse.bass_utils import run_bass_kernel_spmd

    q = np.asarray(q, np.float32)
    k = np.asarray(k, np.float32)
    v = np.asarray(v, np.float32)
    Wq = np.asarray(Wq, np.float32)
    Wk = np.asarray(Wk, np.float32)
    Wv = np.asarray(Wv, np.float32)
    Wo = np.asarray(Wo, np.float32)
    bq = np.asarray(bq, np.float32)
    bv = np.asarray(bv, np.float32)
    bo = np.asarray(bo, np.float32)

    nc = _get_nc()

    sel = np.zeros((64, 128), np.float32)
    sel[0, 0:64] = 1.0
    sel[32, 64:128] = 1.0

    def blk_x(x):
        # x^T [D, S] -> [cb*128+p, kc*512+c] block layout (see dram tensors)
        t = x.T.astype(np.float16).reshape(KCD, 128, 4, 512)
        return np.ascontiguousarray(t.transpose(2, 1, 0, 3).reshape(512, KCD * 512))

    def blk_w(wt):
        # W_gr^T [D, DH] -> [128, kc*DH+c]
        t = wt.astype(np.float16).reshape(KCD, 128, DH)
        return np.ascontiguousarray(t.transpose(1, 0, 2).reshape(128, KCD * DH))

    xT = {}
    for b in range(B):
        xT[("q", b)] = blk_x(q[b])
        xT[("k", b)] = blk_x(k[b])
        xT[("v", b)] = blk_x(v[b])

    in_maps = []
    for c in range(8):
        b, g = c // G, c % G
        gr = slice(g * DH, (g + 1) * DH)
        in_maps.append({
            "xq": xT[("q", b)],
            "xk": xT[("k", b)],
            "xv": xT[("v", b)],
            "wq": blk_w(Wq[gr, :].T),
            "wk": blk_w(Wk[gr, :].T),
            "wv": blk_w(Wv[gr, :].T),
            "wo": np.ascontiguousarray(
                Wo[:, gr].T.astype(np.float16).reshape(2, 128, D)
                .transpose(1, 0, 2).reshape(128, 2 * D)),
            "bq": np.ascontiguousarray(bq[gr].reshape(2, 128).T),
            "sel": sel,
            "zr": np.zeros((64, 512), np.float32),
        })

    kwargs = {}
    if _trace:
        kwargs = dict(trace=True, tmpdir=_tmpdir)
    res = run_bass_kernel_spmd(nc, in_maps, core_ids=list(range(8)), **kwargs)

    # host reduce: y[b] = sum_g y_g^T.T  (+ bias terms folded host-side)
    bias_row = bv @ Wo.T + bo                     # [D]
    out = np.empty((B, S, D), np.float32)
    for b in range(B):
        acc = np.zeros((S, D), np.float32)
        for g in range(G):
            acc += res.results[b * G + g]["y"].T.astype(np.float32)
        out[b] = acc + bias_row[None, :]
    if _trace:
        out = (out, res)
    return out
